# revision 28
# baseline (speedup 1.0000x reference)
"""AttnBlock (GroupNorm + single-head spatial self-attention + residual) on
8 Trainium2 NeuronCores, data-parallel over batch (2 batches per core).

Full inputs in, full outputs out. Per-core Bass/Tile kernel, v2:

  GroupNorm folded into the QKV weights: h = s*x + t  =>
    Q = (wq*s).x + (wq.t + bq)     K = (wk*s).x        V^T = x^T.(wv*s)
  K's additive consts cancel exactly in softmax (per-query shifts);
  V's consts (wv.t + bv) pass through the softmax average exactly and
  fold into the output-projection bias bo'' = wo.(wv.t + bv) + bo.
  x is cast to fp8 once while streaming for stats; all projections are
  fp8 DoubleRow matmuls off the fp8 x copy (weights re-quantized per
  batch after the GroupNorm scale fold).

  S^T   = K^T.Q_chunk              fp8 DR MMs, fp32 PSUM (mt-paired)
  P     = exp(S^T * C^-0.5 - ln16) one [128,1024] EXP per psum pair
  sum_m = pair-add tree (DVE + GpSimd) + ones128 matmul broadcast
  O^T   = V^T.P * (1/s)            fp8 DR MMs, cs-paired PSUM
  out   = wo.O^T + bo'' + x        fp8 DR MMs, residual in fp32

Both batches run as one 16-chunk software pipeline: batch 1's x
stream/stats/weight-fold/QKV are interleaved into batch 0's attention
chunks 1..8, and the AV/OP lag crosses the batch boundary, so TensorE
never drains between batches. Engine assignment keeps the Scalar
engine Exp-only during attention (activation table reloads cost
1.3us): evacuations go to DVE/GpSimd, with the one-off QKV(0) phase
round-robining Scalar (Identity) before the first EXP.
"""

import numpy as np
import ml_dtypes

import concourse.bass as bass
import concourse.tile as tile
from concourse import bacc, mybir
from concourse.bass_utils import run_bass_kernel_spmd

P = 128
C = 512
HW = 4096
NB = 2           # batches per core
NCORES = 8
NCT = C // P     # 4 c-tiles
NPT = 2          # c-pair tiles (256 channels each)
NCH = HW // 512  # 8 q-chunks per batch
NMT = HW // P    # 32 m-tiles
NSLOT = NB * NCH  # 16 chunk slots
G = 32           # groups
GS = C // G      # 16 channels per group
EPS = 1e-5
LN16 = float(np.log(16.0))
ISQC = float(C ** -0.5)

f32 = mybir.dt.float32
bf16 = mybir.dt.bfloat16
fp8 = mybir.dt.float8e4
DR = mybir.MatmulPerfMode.DoubleRow
ADD = mybir.AluOpType.add
MULT = mybir.AluOpType.mult
SUB = mybir.AluOpType.subtract
AF = mybir.ActivationFunctionType


def _build():
    nc = bacc.Bacc("TRN2", target_bir_lowering=False, debug=False,
                   num_devices=NCORES)

    x_d = nc.dram_tensor("x", [NB, C, HW], f32, kind="ExternalInput").ap()
    wq_d = nc.dram_tensor("wq8", [NPT, P, 2, C], fp8, kind="ExternalInput").ap()
    wk_d = nc.dram_tensor("wk8", [NPT, P, 2, C], fp8, kind="ExternalInput").ap()
    wv_d = nc.dram_tensor("wv8", [NPT, P, 2, C], fp8, kind="ExternalInput").ap()
    wo_d = nc.dram_tensor("wo8", [NPT, P, 2, C], fp8, kind="ExternalInput").ap()
    bq_d = nc.dram_tensor("bq", [C], f32, kind="ExternalInput").ap()
    bv_d = nc.dram_tensor("bv", [C], f32, kind="ExternalInput").ap()
    bo_d = nc.dram_tensor("bo", [C], f32, kind="ExternalInput").ap()
    gnw_d = nc.dram_tensor("gnw", [C], f32, kind="ExternalInput").ap()
    gnb_d = nc.dram_tensor("gnb", [C], f32, kind="ExternalInput").ap()
    ag_d = nc.dram_tensor("A_g", [P, 8], f32, kind="ExternalInput").ap()
    as_d = nc.dram_tensor("A_s", [8, P], f32, kind="ExternalInput").ap()
    out_d = nc.dram_tensor("out", [NB, C, HW], f32, kind="ExternalOutput").ap()

    with tile.TileContext(nc) as tc:
        with (
            tc.tile_pool(name="kp", bufs=4) as kp,
            tc.tile_pool(name="qp", bufs=20) as qp,
            tc.tile_pool(name="vt", bufs=32) as vtp,
            tc.tile_pool(name="x8p", bufs=9) as x8p,
            tc.tile_pool(name="work", bufs=34) as work,
            tc.tile_pool(name="tree", bufs=9) as treep,
            tc.tile_pool(name="wpool", bufs=8) as wpool,
            tc.tile_pool(name="wfold", bufs=8) as wfold,
            tc.tile_pool(name="accp", bufs=2) as accp,
            tc.tile_pool(name="xin", bufs=3) as xin,
            tc.tile_pool(name="xres", bufs=3) as xres,
            tc.tile_pool(name="otp", bufs=4) as otp,
            tc.tile_pool(name="rcp", bufs=2) as rcp,
            tc.tile_pool(name="small", bufs=4) as small,
            tc.tile_pool(name="cons", bufs=1) as cons,
            tc.tile_pool(name="ps_s", bufs=3, space="PSUM") as ps_s,
            tc.tile_pool(name="ps_av", bufs=2, space="PSUM") as ps_av,
        ):
            # round-robin evacuation engine chooser
            def evac_rr(b):
                # GpSimd cannot read PSUM; evacs are Scalar/Vector only.
                # b0 phases run before the first EXP: Scalar takes most
                # (DVE must keep capacity for batch-1 stats).
                engs = ([("s", None), ("s", None), ("v", None)] if b == 0
                        else [("v", None)])
                state = {"i": 0}

                def pick():
                    e = engs[state["i"] % len(engs)][0]
                    state["i"] += 1
                    return e
                return pick

            def evac_copy(eng, out, in_):
                if eng == "s":
                    nc.scalar.add(out=out, in_=in_, add=0.0)
                elif eng == "v":
                    nc.vector.tensor_copy(out=out, in_=in_)
                else:
                    nc.gpsimd.tensor_copy(out=out, in_=in_)

            # ---------------- GroupNorm stats + fp8 x copy ----------------
            def load_stats(b, cts, x8, stats):
                """Stream x c-tiles `cts`, cast to fp8 (Scalar for b0,
                GpSimd for b1 -- keeps Scalar Exp-only mid-attention),
                accumulate bn_stats. x8[pt][j2] tiles are [P, 2, 1024]."""
                for ct in cts:
                    pt, s = ct // 2, ct % 2
                    stats_t = small.tile([P, 8, 6], f32, tag="stats",
                                         name=f"st{b}_{ct}")
                    stats[ct] = stats_t
                    for j2 in range(4):
                        xt = xin.tile([P, 1024], f32, tag="xin",
                                      name=f"xs{b}{ct}{j2}")
                        nc.sync.dma_start(
                            out=xt[:],
                            in_=x_d[b, ct * P:(ct + 1) * P,
                                    j2 * 1024:(j2 + 1) * 1024])
                        if s == 0:
                            x8[pt][j2] = x8p.tile([P, 2, 1024], fp8, tag="x8",
                                                  name=f"x8_{b}{pt}{j2}")
                        if b == 0:
                            nc.scalar.add(out=x8[pt][j2][:, s, :], in_=xt[:],
                                          add=0.0)
                        else:
                            # split the cast: DVE half, GpSimd half
                            nc.vector.tensor_copy(
                                out=x8[pt][j2][:, s, 0:512],
                                in_=xt[:, 0:512])
                            nc.gpsimd.tensor_copy(
                                out=x8[pt][j2][:, s, 512:1024],
                                in_=xt[:, 512:1024])
                        for jj in range(2):
                            nc.vector.bn_stats(
                                out=stats_t[:, j2 * 2 + jj, :],
                                in_=xt[:, jj * 512:(jj + 1) * 512])

            def gn_phase2(b, stats):
                """bn_aggr + group-combine via tiny MMs ->
                sb2[ct] [P, 2] = (scale_c, t_c)."""
                sb2s = [None] * NCT
                stat2s = [None] * NCT
                for ct in range(NCT):
                    mv_t = small.tile([P, 2], f32, tag="mv", name=f"mv{b}_{ct}")
                    nc.vector.bn_aggr(out=mv_t[:], in_=stats[ct][:])
                    stat2 = small.tile([P, 2], f32, tag="stat2",
                                       name=f"s2{b}_{ct}")
                    nc.vector.tensor_copy(out=stat2[:, 0:1], in_=mv_t[:, 0:1])
                    nc.vector.tensor_tensor(stat2[:, 1:2], mv_t[:, 0:1],
                                            mv_t[:, 0:1], MULT)
                    nc.vector.tensor_tensor(stat2[:, 1:2], stat2[:, 1:2],
                                            mv_t[:, 1:2], ADD)
                    stat2s[ct] = stat2
                vt2s = [None] * NCT
                gs2s = [None] * NCT
                for ct in range(NCT):
                    gst_ps = ps_s.tile([8, 2], f32, tag="s", name=f"gst{b}{ct}")
                    nc.tensor.matmul(gst_ps[:], ag_t[:], stat2s[ct][:],
                                     start=True, stop=True)
                    gsb = small.tile([8, 2], f32, tag="gsb", name=f"gsb{b}{ct}")
                    nc.vector.tensor_copy(out=gsb[:], in_=gst_ps[:])
                    vt2 = small.tile([8, 2], f32, tag="vt2", name=f"vt2{b}{ct}")
                    nc.vector.tensor_tensor(vt2[:, 0:1], gsb[:, 0:1],
                                            gsb[:, 0:1], MULT)
                    nc.vector.tensor_tensor(vt2[:, 1:2], gsb[:, 1:2],
                                            vt2[:, 0:1], SUB)
                    gs2 = small.tile([8, 2], f32, tag="gs2", name=f"gs2{b}{ct}")
                    nc.vector.tensor_copy(out=gs2[:, 0:1], in_=gsb[:, 0:1])
                    vt2s[ct] = vt2
                    gs2s[ct] = gs2
                # group activations: all Ln, then all Exp (2 table loads)
                for ct in range(NCT):
                    nc.scalar.activation(out=vt2s[ct][:, 0:1],
                                         in_=vt2s[ct][:, 1:2],
                                         func=AF.Ln, bias=eps_t[:8])
                for ct in range(NCT):
                    nc.scalar.activation(out=gs2s[ct][:, 1:2],
                                         in_=vt2s[ct][:, 0:1],
                                         func=AF.Exp, scale=-0.5)
                for ct in range(NCT):
                    cst_ps = ps_s.tile([P, 2], f32, tag="s", name=f"cst{b}{ct}")
                    nc.tensor.matmul(cst_ps[:], as_t[:], gs2s[ct][:],
                                     start=True, stop=True)
                    sb2 = small.tile([P, 2], f32, tag="sb2", name=f"sb2{b}{ct}")
                    nc.vector.tensor_tensor(sb2[:, 0:1], cst_ps[:, 1:2],
                                            gnw4[:, ct:ct + 1], MULT)
                    nc.vector.tensor_tensor(sb2[:, 1:2], cst_ps[:, 0:1],
                                            sb2[:, 0:1], MULT)
                    nc.vector.tensor_tensor(sb2[:, 1:2], gnb4[:, ct:ct + 1],
                                            sb2[:, 1:2], SUB)
                    sb2s[ct] = sb2
                return sb2s

            def fold_w(b, bs, sb2s, key, src):
                """Fold GN scale into one projection's weights (DVE)."""
                wf = [wfold.tile([P, 2, C], fp8, tag="wf",
                                 name=f"{key}{b}{pt}") for pt in range(NPT)]
                for pt in range(NPT):
                    for s in range(2):
                        nc.vector.tensor_scalar_mul(
                            wf[pt][:, s, :], src[pt][:, s, :],
                            sb2s[2 * pt + s][:, 0:1])
                bs[key] = wf

            def fold_t(b, bs, sb2s):
                """t16 pair tiles: t16[pt][p, s, 0] = 16*t_{pt*256+s*128+p}"""
                t16 = [small.tile([P, 2, 1], fp8, tag="t16",
                                  name=f"t16_{b}{pt}") for pt in range(NPT)]
                for pt in range(NPT):
                    for s in range(2):
                        nc.vector.tensor_scalar_mul(
                            t16[pt][:, s, :], sb2s[2 * pt + s][:, 1:2], 16.0)
                bs["t16"] = t16

            def fold_consts(b, bs):
                """Derived bias consts bq_eff [P,NCT], bo_eff [P,NCT]
                (uses original weights + t16 only)."""
                t16 = bs["t16"]
                # dq = wq.t (unfolded wq), per c_out column layout [P, NCT]
                dq_ps = ps_s.tile([P, NCT], f32, tag="s", name=f"dq{b}")
                for ct in range(NCT):
                    csl = slice(ct * P, (ct + 1) * P)
                    for pt in range(NPT):
                        nc.tensor.matmul(dq_ps[:, ct:ct + 1],
                                         wq8[pt][:, :, csl], t16[pt][:],
                                         start=(pt == 0), stop=(pt == 1),
                                         perf_mode=DR)
                bq_eff = small.tile([P, NCT], f32, tag="bqe", name=f"bqe{b}")
                nc.vector.scalar_tensor_tensor(
                    out=bq_eff[:], in0=dq_ps[:], scalar=1.0 / 16.0,
                    in1=bq4[:], op0=MULT, op1=ADD)
                # dv' = wv.t in column layout, then dvbv = dv' + bv
                dv_ps = ps_s.tile([P, NCT], f32, tag="s", name=f"dv{b}")
                for ct in range(NCT):
                    csl = slice(ct * P, (ct + 1) * P)
                    for pt in range(NPT):
                        nc.tensor.matmul(dv_ps[:, ct:ct + 1],
                                         wv8[pt][:, :, csl], t16[pt][:],
                                         start=(pt == 0), stop=(pt == 1),
                                         perf_mode=DR)
                dvbv = small.tile([P, NCT], f32, tag="dvbv", name=f"dvbv{b}")
                nc.vector.scalar_tensor_tensor(
                    out=dvbv[:], in0=dv_ps[:], scalar=1.0 / 16.0,
                    in1=bv4[:], op0=MULT, op1=ADD)
                # pair-ize 16*(dv'+bv) for the wo matvec
                dvp = [small.tile([P, 2, 1], fp8, tag="dvp",
                                  name=f"dvp{b}{pt}") for pt in range(NPT)]
                for pt in range(NPT):
                    for s in range(2):
                        nc.vector.tensor_scalar_mul(
                            dvp[pt][:, s, :],
                            dvbv[:, 2 * pt + s:2 * pt + s + 1], 16.0)
                dbo_ps = ps_s.tile([P, NCT], f32, tag="s", name=f"dbo{b}")
                for ct in range(NCT):
                    csl = slice(ct * P, (ct + 1) * P)
                    for pt in range(NPT):
                        nc.tensor.matmul(dbo_ps[:, ct:ct + 1],
                                         wo8[pt][:, :, csl], dvp[pt][:],
                                         start=(pt == 0), stop=(pt == 1),
                                         perf_mode=DR)
                bo_eff = small.tile([P, NCT], f32, tag="boe", name=f"boe{b}")
                nc.vector.scalar_tensor_tensor(
                    out=bo_eff[:], in0=dbo_ps[:], scalar=1.0 / 16.0,
                    in1=bo4[:], op0=MULT, op1=ADD)
                bs["bq_eff"], bs["bo_eff"] = bq_eff, bo_eff

            # ---------------- QKV projections ----------------
            def qkv_k(b, bs, ns):
                """K projection for chunks ns: pure copies out (no bias)."""
                for n in ns:
                    nsl = slice(n * 512, (n + 1) * 512)
                    x8n = bs["x8"]
                    rhs_j2, rhs_h = n // 2, n % 2
                    for opt in range(NPT):
                        k_ps = ps_s.tile([P, 2, 512], f32, tag="s",
                                         name=f"kps{b}{n}{opt}")
                        for s in range(2):
                            csl = slice((2 * opt + s) * P,
                                        (2 * opt + s + 1) * P)
                            for pt in range(NPT):
                                nc.tensor.matmul(
                                    k_ps[:, s, :],
                                    bs["wkf"][pt][:, :, csl],
                                    x8n[pt][rhs_j2][
                                        :, :, rhs_h * 512:(rhs_h + 1) * 512],
                                    start=(pt == 0), stop=(pt == 1),
                                    perf_mode=DR)
                        evac_copy(bs["rr"](), bs["k8"][opt][:, :, nsl],
                                  k_ps[:])

            def qkv_q(b, bs, ns):
                """Q projection for chunks ns with bias bq_eff."""
                for n in ns:
                    x8n = bs["x8"]
                    rhs_j2, rhs_h = n // 2, n % 2
                    for opt in range(NPT):
                        q_ps = ps_s.tile([P, 2, 512], f32, tag="s",
                                         name=f"qps{b}{n}{opt}")
                        for s in range(2):
                            csl = slice((2 * opt + s) * P,
                                        (2 * opt + s + 1) * P)
                            for pt in range(NPT):
                                nc.tensor.matmul(
                                    q_ps[:, s, :],
                                    bs["wqf"][pt][:, :, csl],
                                    x8n[pt][rhs_j2][
                                        :, :, rhs_h * 512:(rhs_h + 1) * 512],
                                    start=(pt == 0), stop=(pt == 1),
                                    perf_mode=DR)
                        q8 = qp.tile([P, 2, 512], fp8, tag="q8",
                                     name=f"q8_{b}_{n}_{opt}")
                        bs["q8"][n][opt] = q8
                        for s in range(2):
                            ct = 2 * opt + s
                            if b == 0:
                                nc.scalar.add(out=q8[:, s, :],
                                              in_=q_ps[:, s, :],
                                              add=bs["bq_eff"][:, ct:ct + 1])
                            else:
                                nc.vector.tensor_scalar_add(
                                    q8[:, s, :], q_ps[:, s, :],
                                    bs["bq_eff"][:, ct:ct + 1])

            def qkv_v(b, bs, mt2s):
                """V^T projection, mt2 (pair) granularity."""
                for mt2 in mt2s:
                    v_ps = ps_s.tile([P, 2, 512], f32, tag="s",
                                     name=f"vps{b}{mt2}")
                    for h in range(2):
                        mt = 2 * mt2 + h
                        j2, sub = mt // 8, mt % 8
                        for pt in range(NPT):
                            nc.tensor.matmul(
                                v_ps[:, h, :],
                                bs["x8"][pt][j2][:, :, sub * P:(sub + 1) * P],
                                bs["wvf"][pt][:],
                                start=(pt == 0), stop=(pt == 1),
                                perf_mode=DR)
                    v8 = vtp.tile([P, 2, 512], fp8, tag="vt",
                                  name=f"v{b}_{mt2}")
                    bs["v8"][mt2] = v8
                    evac_copy(bs["rr"](), v8[:], v_ps[:])

            # ---------------- attention chunk slot ----------------
            class Pipe:
                pass

            pipe = Pipe()
            pipe.p_prev = None      # P tiles of chunk g-1
            pipe.acc_prev = None    # softmax denominator acc of chunk g-1
            pipe.recip = None       # recip of chunk g-1 (made early slot g)
            pipe.ot8 = None         # [ot8_pt0, ot8_pt1] for chunk g-2
            pipe.ot8_next = None    # same, for chunk g-1 (filled this slot)
            pipe.av_ps = None       # live AV psum bank

            def emit_recip(g):
                sb_ps = ps_s.tile([P, 2, 512], f32, tag="s", name=f"sbps{g}")
                nc.tensor.matmul(sb_ps[:, 0, :], ones128[:], pipe.acc_prev[:],
                                 start=True, stop=True)
                rt = rcp.tile([P, 512], f32, tag="recip", name=f"rt{g % 2}")
                nc.vector.reciprocal_approx_fast(out=rt[:], in_=sb_ps[:, 0, :])
                pipe.recip = rt

            def emit_op(g, half):
                """Output proj + bias + residual (in-place) + store for
                chunk g-2."""
                c = g - 2
                b, ic = c // NCH, c % NCH
                bs = bstate[b]
                qsl = slice(ic * 512, (ic + 1) * 512)
                op_ps = ps_s.tile([P, 2, 512], f32, tag="s",
                                  name=f"op{c}_{half}")
                for hh in range(2):
                    ct = half * 2 + hh
                    csl = slice(ct * P, (ct + 1) * P)
                    for pt in range(NPT):
                        nc.tensor.matmul(
                            op_ps[:, hh, :], wo8[pt][:, :, csl],
                            pipe.ot8[pt][:],
                            start=(pt == 0), stop=(pt == 1), perf_mode=DR)
                xr = bs["xr"][half]
                for hh in range(2):
                    ct = half * 2 + hh
                    nc.vector.scalar_tensor_tensor(
                        out=xr[:, hh, :], in0=op_ps[:, hh, :],
                        scalar=bs["bo_eff"][:, ct:ct + 1], in1=xr[:, hh, :],
                        op0=ADD, op1=ADD)
                for hh in range(2):
                    ct = half * 2 + hh
                    nc.sync.dma_start(
                        out=out_d[b, ct * P:(ct + 1) * P, qsl],
                        in_=xr[:, hh, :])

            def chunk(g, hooks=None):
                """Slot g: S/EXP/sum-tree for chunk g (if g<16); AV for
                chunk g-1; recip for g-1; OP for g-2. hooks: dict of
                mt -> callable, extra work woven into the stream."""
                do_s = g < NSLOT
                do_av = 1 <= g <= NSLOT
                do_op = 2 <= g
                b = g // NCH if do_s else None
                bs = bstate[b] if do_s else None
                i = g % NCH if do_s else 0
                bp = (g - 1) // NCH if do_av else None
                bsp = bstate[bp] if do_av else None

                # prefetch residual x for chunk g-2's OP
                if do_op:
                    c = g - 2
                    bo_, ico = c // NCH, c % NCH
                    qsl = slice(ico * 512, (ico + 1) * 512)
                    xrs = []
                    for half in range(2):
                        xr = xres.tile([P, 2, 512], f32, tag="xres",
                                       name=f"xr{c}_{half}")
                        for hh in range(2):
                            ct = half * 2 + hh
                            nc.sync.dma_start(
                                out=xr[:, hh, :],
                                in_=x_d[bo_, ct * P:(ct + 1) * P, qsl])
                        xrs.append(xr)
                    bstate[bo_]["xr"] = xrs
                    pipe.ot8 = pipe.ot8_next

                # AV for chunk g-1: two cs-PAIR passes, each alternating
                # between two single-bank PSUM tiles (back-to-back
                # accumulation into one bank runs at half rate); evac each
                # cs right at its stop.
                n_av = 0

                def emit_av():
                    nonlocal n_av
                    if not do_av or n_av >= 2 * NMT:
                        return
                    pair = n_av // NMT
                    idx = n_av % NMT
                    mt2, h = idx // 2, idx % 2
                    cs = 2 * pair + h
                    if idx == 0:
                        pipe.av_ps = [
                            ps_av.tile([P, 512], f32, tag="av",
                                       name=f"av{g}_{2 * pair + hh}")
                            for hh in range(2)]
                        if pair == 0:
                            pipe.ot8_next = [
                                otp.tile([P, 2, 512], fp8, tag="ot",
                                         name=f"ot{g - 1}_{pt}")
                                for pt in range(NPT)]
                    nc.tensor.matmul(
                        pipe.av_ps[h][:],
                        bsp["v8"][mt2][:, :, cs * P:(cs + 1) * P],
                        pipe.p_prev[mt2][:],
                        start=(mt2 == 0), stop=(mt2 == NMT // 2 - 1),
                        perf_mode=DR)
                    n_av += 1
                    if idx >= NMT - 2:
                        nc.vector.tensor_tensor(
                            pipe.ot8_next[pair][:, h, :],
                            pipe.av_ps[h][:], pipe.recip[:], MULT)

                # in-place pair-add tree over the 16 P pair-tiles
                p_cur = [None] * (NMT // 2) if do_s else None
                tt = [None] * 8

                def tree_l1(j):
                    t = treep.tile([P, 2, 512], bf16, tag="tr",
                                   name=f"t{g}_{j}")
                    eng = nc.gpsimd if j % 2 == 0 else nc.vector
                    eng.tensor_tensor(t[:], p_cur[2 * j][:],
                                      p_cur[2 * j + 1][:], ADD)
                    tt[j] = t

                def tree_join(dst, src):
                    nc.vector.tensor_tensor(tt[dst][:], tt[dst][:],
                                            tt[src][:], ADD)

                if not do_s and do_av and pipe.acc_prev is not None:
                    emit_recip(g)

                for mt in range(NMT if do_s else 8):
                    if do_s:
                        mt2 = mt // 2
                        if mt % 2 == 0:
                            pipe.s_ps = ps_s.tile([P, 2, 512], f32, tag="s",
                                                  name=f"sps{g}_{mt2}")
                        for pt in range(NPT):
                            nc.tensor.matmul(
                                pipe.s_ps[:, mt % 2, :],
                                bs["k8"][pt][:, :, mt * P:(mt + 1) * P],
                                bs["q8"][i][pt][:],
                                start=(pt == 0), stop=(pt == 1),
                                perf_mode=DR)
                        if mt % 2 == 1:
                            p_cur[mt2] = work.tile([P, 2, 512], fp8,
                                                   tag="work",
                                                   name=f"p{g}_{mt2}")
                            nc.scalar.activation(
                                out=p_cur[mt2][:], in_=pipe.s_ps[:],
                                func=AF.Exp, bias=nln16_t[:], scale=ISQC)
                            if mt2 % 2 == 1:
                                tree_l1(mt2 // 2)
                            if mt2 == 3:
                                tree_join(1, 0)
                            elif mt2 == 7:
                                tree_join(3, 2)
                                tree_join(3, 1)
                            elif mt2 == 11:
                                tree_join(5, 4)
                            elif mt2 == 15:
                                tree_join(7, 6)
                                tree_join(7, 5)
                                tree_join(7, 3)
                    if do_s and mt == 8 and pipe.acc_prev is not None:
                        emit_recip(g)
                    if do_av and mt >= 3:
                        emit_av()
                        emit_av()
                        if mt % 4 == 0:
                            emit_av()
                    if do_op and do_s and mt == 6:
                        emit_op(g, 0)
                    if do_op and do_s and mt == 10:
                        emit_op(g, 1)
                    if hooks and mt in hooks:
                        hooks[mt]()
                while do_av and n_av < 2 * NMT:
                    emit_av()
                if do_op and not do_s:
                    emit_op(g, 0)
                    emit_op(g, 1)

                if do_s:
                    acc = accp.tile([P, 512], bf16, tag="acc",
                                    name=f"acc{g % 2}")
                    nc.vector.tensor_tensor(acc[:], tt[7][:, 0, :],
                                            tt[7][:, 1, :], ADD)
                    pipe.acc_prev = acc
                # roll pipeline state
                pipe.p_prev = p_cur

            # ================= emission schedule =================
            bstate = [{"x8": [[None] * 4 for _ in range(NPT)],
                       "k8": None, "q8": [[None] * NPT for _ in range(NCH)],
                       "v8": [None] * (NMT // 2)} for _ in range(NB)]
            bstate[0]["rr"] = evac_rr(0)
            bstate[1]["rr"] = evac_rr(1)
            stats0 = [None] * NCT
            stats1 = [None] * NCT
            # batch-0 x stream first (leads DMA queues)
            load_stats(0, range(NCT), bstate[0]["x8"], stats0)

            # ---- constants ----
            bq4 = cons.tile([P, NCT], f32, tag="bq4")
            nc.sync.dma_start(out=bq4[:], in_=bq_d.rearrange("(t p) -> p t", p=P))
            bv4 = cons.tile([P, NCT], f32, tag="bv4")
            nc.sync.dma_start(out=bv4[:], in_=bv_d.rearrange("(t p) -> p t", p=P))
            bo4 = cons.tile([P, NCT], f32, tag="bo4")
            nc.sync.dma_start(out=bo4[:], in_=bo_d.rearrange("(t p) -> p t", p=P))
            gnw4 = cons.tile([P, NCT], f32, tag="gnw4")
            nc.sync.dma_start(out=gnw4[:], in_=gnw_d.rearrange("(t p) -> p t", p=P))
            gnb4 = cons.tile([P, NCT], f32, tag="gnb4")
            nc.sync.dma_start(out=gnb4[:], in_=gnb_d.rearrange("(t p) -> p t", p=P))
            ones128 = cons.tile([P, P], bf16, tag="ones128")
            nc.vector.memset(ones128[:], 1.0)
            eps_t = cons.tile([P, 1], f32, tag="eps")
            nc.vector.memset(eps_t[:], EPS)
            nln16_t = cons.tile([P, 1], f32, tag="nln16")
            nc.vector.memset(nln16_t[:], -LN16)
            ag_t = cons.tile([P, 8], f32, tag="ag")
            nc.sync.dma_start(out=ag_t[:], in_=ag_d[:])
            as_t = cons.tile([8, P], f32, tag="as")
            nc.sync.dma_start(out=as_t[:], in_=as_d[:])

            # weights (original, resident for both batches)
            wq8 = [wpool.tile([P, 2, C], fp8, tag="w8", name=f"wq8_{pt}")
                   for pt in range(NPT)]
            wk8 = [wpool.tile([P, 2, C], fp8, tag="w8", name=f"wk8_{pt}")
                   for pt in range(NPT)]
            wv8 = [wpool.tile([P, 2, C], fp8, tag="w8", name=f"wv8_{pt}")
                   for pt in range(NPT)]
            wo8 = [wpool.tile([P, 2, C], fp8, tag="w8", name=f"wo8_{pt}")
                   for pt in range(NPT)]
            for pt in range(NPT):
                nc.sync.dma_start(out=wq8[pt][:], in_=wq_d[pt])
                nc.sync.dma_start(out=wk8[pt][:], in_=wk_d[pt])
                nc.sync.dma_start(out=wv8[pt][:], in_=wv_d[pt])
                nc.sync.dma_start(out=wo8[pt][:], in_=wo_d[pt])

            # batch 0: phase2 + fold + full QKV (pre-attention)
            b0 = bstate[0]
            sb2_0 = gn_phase2(0, stats0)
            fold_w(0, b0, sb2_0, "wkf", wk8)
            b0["k8"] = [kp.tile([P, 2, HW], fp8, tag="k8",
                                name=f"k8_0_{opt}") for opt in range(NPT)]
            qkv_k(0, b0, [0, 1, 2])
            fold_w(0, b0, sb2_0, "wqf", wq8)
            fold_t(0, b0, sb2_0)
            fold_consts(0, b0)
            qkv_k(0, b0, range(3, NCH))
            qkv_q(0, b0, [0, 1])
            fold_w(0, b0, sb2_0, "wvf", wv8)
            qkv_v(0, b0, range(NMT // 2))
            qkv_q(0, b0, range(2, NCH))
            # batch-1 x stream + stats: DMA and DVE are free while
            # QKV(0) occupies TensorE
            load_stats(1, range(NCT), bstate[1]["x8"], stats1)

            # batch-1 work woven into batch-0/boundary chunks
            sb2_1_box = [None]

            def hook_b1(g):
                b1 = bstate[1]
                if g == 0:
                    def ph2():
                        sb2_1_box[0] = gn_phase2(1, stats1)
                        b1["k8"] = [kp.tile([P, 2, HW], fp8, tag="k8",
                                            name=f"k8_1_{opt}")
                                    for opt in range(NPT)]
                    return {14: ph2,
                            24: lambda: fold_w(1, b1, sb2_1_box[0],
                                               "wkf", wk8)}
                if g == 1:
                    return {6: lambda: fold_w(1, b1, sb2_1_box[0],
                                              "wqf", wq8),
                            14: lambda: (fold_w(1, b1, sb2_1_box[0],
                                                "wvf", wv8),
                                         fold_t(1, b1, sb2_1_box[0])),
                            22: lambda: fold_consts(1, b1)}
                if g == 2:
                    return {6: lambda: qkv_k(1, b1, [0, 1]),
                            22: lambda: qkv_k(1, b1, [2, 3])}
                if g == 3:
                    return {6: lambda: qkv_k(1, b1, [4, 5]),
                            22: lambda: qkv_k(1, b1, [6, 7])}
                if g == 4:
                    return {6: lambda: qkv_q(1, b1, [0]),
                            14: lambda: qkv_v(1, b1, range(0, 4)),
                            26: lambda: qkv_q(1, b1, [1])}
                if g == 5:
                    return {6: lambda: qkv_q(1, b1, [2]),
                            14: lambda: qkv_v(1, b1, range(4, 8)),
                            26: lambda: qkv_q(1, b1, [3])}
                if g == 6:
                    return {6: lambda: qkv_q(1, b1, [4]),
                            14: lambda: qkv_v(1, b1, range(8, 12)),
                            26: lambda: qkv_q(1, b1, [5])}
                if g == 7:
                    return {6: lambda: qkv_v(1, b1, range(12, 16)),
                            18: lambda: qkv_q(1, b1, [6])}
                if g == 8:
                    return {6: lambda: qkv_q(1, b1, [7])}
                return None

            for g in range(NSLOT + 2):
                chunk(g, hooks=hook_b1(g))

    nc.finalize()
    return nc


_NC = None


def _program():
    global _NC
    if _NC is None:
        _NC = _build()
    return _NC


def _pair_interleave(wT):
    """[512, 512] (rows = c_in) -> [2, 128, 2, 512] DoubleRow layout:
    out[pt, p, s, :] = wT[pt*256 + s*128 + p, :]"""
    return np.ascontiguousarray(
        wT.reshape(2, 2, P, C).transpose(0, 2, 1, 3))


def _host_prep(inputs):
    x = np.asarray(inputs["x"], np.float32)
    e4 = ml_dtypes.float8_e4m3
    wq8 = _pair_interleave(np.asarray(inputs["wq"], np.float32).T).astype(e4)
    wk8 = _pair_interleave(np.asarray(inputs["wk"], np.float32).T).astype(e4)
    wv8 = _pair_interleave(np.asarray(inputs["wv"], np.float32).T).astype(e4)
    wo8 = _pair_interleave(np.asarray(inputs["wo"], np.float32).T).astype(e4)
    A_g = np.zeros((P, 8), np.float32)
    A_s = np.zeros((8, P), np.float32)
    for p in range(P):
        A_g[p, p // GS] = 1.0 / GS
        A_s[p // GS, p] = 1.0
    shared = {
        "wq8": wq8, "wk8": wk8, "wv8": wv8, "wo8": wo8,
        "bq": np.asarray(inputs["bq"], np.float32),
        "bv": np.asarray(inputs["bv"], np.float32),
        "bo": np.asarray(inputs["bo"], np.float32),
        "gnw": np.asarray(inputs["gn_weight"], np.float32),
        "gnb": np.asarray(inputs["gn_bias"], np.float32),
        "A_g": A_g, "A_s": A_s,
    }
    in_maps = []
    for i in range(NCORES):
        xi = np.ascontiguousarray(
            x[i * NB:(i + 1) * NB].reshape(NB, C, HW), np.float32)
        in_maps.append({"x": xi, **shared})
    return in_maps


def _execute(inputs, trace=False):
    nc = _program()
    in_maps = _host_prep(inputs)
    res = run_bass_kernel_spmd(nc, in_maps, core_ids=list(range(NCORES)),
                               trace=trace)
    outs = [res.results[i]["out"].reshape(NB, C, 64, 64) for i in range(NCORES)]
    out = np.concatenate(outs, axis=0).astype(np.float32)
    return out, res


def kernel(**inputs) -> np.ndarray:
    out, _ = _execute(inputs, trace=False)
    return out
